# revision 4
# baseline (speedup 1.0000x reference)
"""Trainium2 Bass kernel for AlignedLinear (irreps 0e+1o+2e, mul 128).

y[n, o*9+m] = alpha * sum_i x[n, i*9+m] * K[irrep(m), i, o]

Strategy (data-parallel over nodes, 8 cores):
  - pad 50000 nodes -> 8 * 6656, each core gets 13 tiles of 512 nodes
  - per 512-node tile: load x naturally [128n x 1152], PE-transpose the 9
    strided m-slices to xT_m [128i x 512n] (fp32, exact), then for each
    (m, j-block) one fp32 matmul with xT as the stationary operand:
    out = xT_block^T @ K_irrep(m) = y_block [128n x 128o] (n-major, no
    second transpose needed), finally strided scaled copies assemble the
    interleaved [n, (o,m)] output tile and contiguous DMAs store it.
"""

import os

import numpy as np

N_NODES = 50000
DIM = 1152
MUL = 128
NMDIM = 9  # total irrep dim (1+3+5)
IRREP_OF_M = [0, 1, 1, 1, 2, 2, 2, 2, 2]
ALPHA = float(np.sqrt(1.0 / MUL))
N_CORES = 8
TILE_N = 512
TILES_PER_CORE = 13
PER_CORE = TILE_N * TILES_PER_CORE  # 6656
M_GROUPS = [[0, 1, 2, 3], [4, 5, 6, 7], [8]]

_cache = {}

LAST_RESULTS = None  # BassKernelResults of the most recent run (for test.py)


def _install_trace_support():
    """Make trace=True work under axon: inject the missing
    antenv.axon_hooks module and neuter the S3 artifact upload."""
    import contextlib
    import ctypes
    import sys
    import types

    if "antenv.axon_hooks" not in sys.modules:
        mod = types.ModuleType("antenv.axon_hooks")

        def _make_hook():
            try:
                lib = ctypes.CDLL("/opt/axon/libaxon_pjrt.so")
            except OSError:
                return None
            if not hasattr(lib, "axon_start_nrt_profile"):
                return None
            lib.axon_start_nrt_profile.argtypes = [
                ctypes.POINTER(ctypes.c_int64),
                ctypes.c_size_t,
            ]
            lib.axon_start_nrt_profile.restype = ctypes.c_int64
            lib.axon_stop_nrt_profile.argtypes = [ctypes.c_char_p]
            lib.axon_stop_nrt_profile.restype = ctypes.c_int64

            @contextlib.contextmanager
            def _hook(output_dir, device_ids):
                import jax

                jax.devices()
                if device_ids:
                    ids = (ctypes.c_int64 * len(device_ids))(*device_ids)
                    rc = lib.axon_start_nrt_profile(ids, len(device_ids))
                else:
                    rc = lib.axon_start_nrt_profile(None, 0)
                if rc != 0:
                    raise RuntimeError(f"axon_start_nrt_profile rc={rc}")
                try:
                    yield
                finally:
                    n = lib.axon_stop_nrt_profile(str(output_dir).encode())
                    print(f"ntff profile: {n} file(s) -> {output_dir}")

            return _hook

        hook = _make_hook()
        mod.get_axon_ntff_profile_hook = lambda: hook
        mod.set_axon_ntff_profile_hook = lambda h: None
        sys.modules["antenv.axon_hooks"] = mod

    import concourse.bass_utils as bass_utils

    bass_utils.upload_artifacts = lambda tmpdir: tmpdir


def _build():
    import concourse.mybir as mybir
    import concourse.tile as tile
    from concourse import bacc
    from concourse.masks import make_identity

    F32 = mybir.dt.float32

    nc = bacc.Bacc("TRN2", target_bir_lowering=False)
    x = nc.dram_tensor("x", [PER_CORE, DIM], F32, kind="ExternalInput").ap()
    kern = nc.dram_tensor("kern", [3, MUL, MUL], F32, kind="ExternalInput").ap()
    y = nc.dram_tensor("y", [PER_CORE, DIM], F32, kind="ExternalOutput").ap()

    with tile.TileContext(nc) as tc:
        with (
            tc.tile_pool(name="const", bufs=1) as const_pool,
            tc.tile_pool(name="xin", bufs=3) as xin_pool,
            tc.tile_pool(name="xts", bufs=12) as xts_pool,
            tc.tile_pool(name="yout", bufs=3) as yout_pool,
            tc.tile_pool(name="xtp", bufs=2, space="PSUM") as xtp_pool,
            tc.tile_pool(name="yps", bufs=2, space="PSUM") as yps_pool,
        ):
            ident = const_pool.tile([MUL, MUL], F32)
            make_identity(nc, ident[:])
            # kern_sb: [i=128, (t,o)=384], pre-scaled by alpha
            kern_sb = const_pool.tile([MUL, 3 * MUL], F32)
            nc.sync.dma_start(
                out=kern_sb[:].rearrange("i (t o) -> i t o", t=3),
                in_=kern.rearrange("t i o -> i t o"),
            )
            nc.vector.tensor_scalar_mul(kern_sb[:], kern_sb[:], ALPHA)

            for t in range(TILES_PER_CORE):
                nbase = t * TILE_N
                x_sb = []
                for j in range(4):
                    xt_in = xin_pool.tile([MUL, DIM], F32, tag=f"xin{j}")
                    nc.sync.dma_start(
                        out=xt_in[:],
                        in_=x[nbase + j * MUL : nbase + (j + 1) * MUL, :],
                    )
                    x_sb.append(xt_in)

                # transpose all 9 m-slices: xT_m [i=128, n=512]
                xt_all = []
                for m in range(NMDIM):
                    xtp = xtp_pool.tile([MUL, TILE_N], F32, tag="xtp")
                    for j in range(4):
                        nc.tensor.transpose(
                            xtp[:, j * MUL : (j + 1) * MUL],
                            x_sb[j][:].rearrange("p (i m) -> p m i", m=NMDIM)[
                                :, m, :
                            ],
                            ident[:],
                        )
                    xt_sb = xts_pool.tile([MUL, TILE_N], F32, tag="xts")
                    if m % 2 == 0:
                        nc.vector.tensor_copy(xt_sb[:], xtp[:])
                    else:
                        nc.scalar.copy(xt_sb[:], xtp[:])
                    xt_all.append(xt_sb)

                # per n-block: 9 matmuls into one 3-bank PSUM tensor, then
                # a single strided assembly copy and the store DMA
                for j in range(4):
                    yp = yps_pool.tile([MUL, NMDIM * MUL], F32, tag="yps")
                    for m in range(NMDIM):
                        ks = IRREP_OF_M[m] * MUL
                        nc.tensor.matmul(
                            yp[:, m * MUL : (m + 1) * MUL],
                            xt_all[m][:, j * MUL : (j + 1) * MUL],
                            kern_sb[:, ks : ks + MUL],
                            start=True,
                            stop=True,
                        )
                    out_sb = yout_pool.tile(
                        [MUL, DIM], F32, tag=f"yout{j}", name=f"yout{j}"
                    )
                    dst = out_sb[:].rearrange("p (o m) -> p m o", m=NMDIM)
                    src = yp[:].rearrange("p (m o) -> p m o", m=NMDIM)
                    if j % 2 == 0:
                        nc.vector.tensor_copy(dst, src)
                    else:
                        nc.scalar.copy(dst, src)
                    nc.sync.dma_start(
                        out=y[nbase + j * MUL : nbase + (j + 1) * MUL, :],
                        in_=out_sb[:],
                    )

    nc.compile()
    return nc


def kernel(**inputs):
    from concourse import bass_utils

    x = np.ascontiguousarray(np.asarray(inputs["x"], dtype=np.float32))
    kern = np.ascontiguousarray(np.asarray(inputs["kernel"], dtype=np.float32))

    if "nc" not in _cache:
        _cache["nc"] = _build()
    nc = _cache["nc"]

    xp = np.zeros((N_CORES * PER_CORE, DIM), dtype=np.float32)
    xp[:N_NODES] = x
    in_maps = [
        {"x": xp[c * PER_CORE : (c + 1) * PER_CORE], "kern": kern}
        for c in range(N_CORES)
    ]

    trace = os.environ.get("KERNEL_TRACE", "0") == "1"
    if trace:
        _install_trace_support()
    res = bass_utils.run_bass_kernel_spmd(
        nc, in_maps, core_ids=list(range(N_CORES)), trace=trace
    )
    global LAST_RESULTS
    LAST_RESULTS = res

    out = np.concatenate([res.results[c]["y"] for c in range(N_CORES)], axis=0)
    return np.ascontiguousarray(out[:N_NODES])


# revision 5
# speedup vs baseline: 1.2818x; 1.2818x over previous
"""Trainium2 Bass kernel for AlignedLinear (irreps 0e+1o+2e, mul 128).

y[n, o*9+m] = alpha * sum_i x[n, i*9+m] * K[irrep(m), i, o]

Strategy (data-parallel over nodes, 8 cores):
  - pad 50000 nodes -> 8 * 6656, each core gets 13 tiles of 512 nodes
  - per 512-node tile: load x naturally [128n x 1152], PE-transpose the 9
    strided m-slices to xT_m [128i x 512n] (fp32, exact), then for each
    (m, j-block) one fp32 matmul with xT as the stationary operand:
    out = xT_block^T @ K_irrep(m) = y_block [128n x 128o] (n-major, no
    second transpose needed), finally strided scaled copies assemble the
    interleaved [n, (o,m)] output tile and contiguous DMAs store it.
"""

import os

import numpy as np

N_NODES = 50000
DIM = 1152
MUL = 128
NMDIM = 9  # total irrep dim (1+3+5)
IRREP_OF_M = [0, 1, 1, 1, 2, 2, 2, 2, 2]
ALPHA = float(np.sqrt(1.0 / MUL))
N_CORES = 8
TILE_N = 512
TILES_PER_CORE = 13
PER_CORE = TILE_N * TILES_PER_CORE  # 6656
M_GROUPS = [[0, 1, 2, 3], [4, 5, 6, 7], [8]]

_cache = {}

LAST_RESULTS = None  # BassKernelResults of the most recent run (for test.py)


def _install_trace_support():
    """Make trace=True work under axon: inject the missing
    antenv.axon_hooks module and neuter the S3 artifact upload."""
    import contextlib
    import ctypes
    import sys
    import types

    if "antenv.axon_hooks" not in sys.modules:
        mod = types.ModuleType("antenv.axon_hooks")

        def _make_hook():
            try:
                lib = ctypes.CDLL("/opt/axon/libaxon_pjrt.so")
            except OSError:
                return None
            if not hasattr(lib, "axon_start_nrt_profile"):
                return None
            lib.axon_start_nrt_profile.argtypes = [
                ctypes.POINTER(ctypes.c_int64),
                ctypes.c_size_t,
            ]
            lib.axon_start_nrt_profile.restype = ctypes.c_int64
            lib.axon_stop_nrt_profile.argtypes = [ctypes.c_char_p]
            lib.axon_stop_nrt_profile.restype = ctypes.c_int64

            @contextlib.contextmanager
            def _hook(output_dir, device_ids):
                import jax

                jax.devices()
                if device_ids:
                    ids = (ctypes.c_int64 * len(device_ids))(*device_ids)
                    rc = lib.axon_start_nrt_profile(ids, len(device_ids))
                else:
                    rc = lib.axon_start_nrt_profile(None, 0)
                if rc != 0:
                    raise RuntimeError(f"axon_start_nrt_profile rc={rc}")
                try:
                    yield
                finally:
                    n = lib.axon_stop_nrt_profile(str(output_dir).encode())
                    print(f"ntff profile: {n} file(s) -> {output_dir}")

            return _hook

        hook = _make_hook()
        mod.get_axon_ntff_profile_hook = lambda: hook
        mod.set_axon_ntff_profile_hook = lambda h: None
        sys.modules["antenv.axon_hooks"] = mod

    import concourse.bass_utils as bass_utils

    bass_utils.upload_artifacts = lambda tmpdir: tmpdir


def _build():
    import concourse.mybir as mybir
    import concourse.tile as tile
    from concourse import bacc
    from concourse.masks import make_identity

    F32 = mybir.dt.float32

    nc = bacc.Bacc("TRN2", target_bir_lowering=False)
    x = nc.dram_tensor("x", [PER_CORE, DIM], F32, kind="ExternalInput").ap()
    kern = nc.dram_tensor("kern", [3, MUL, MUL], F32, kind="ExternalInput").ap()
    y = nc.dram_tensor("y", [PER_CORE, DIM], F32, kind="ExternalOutput").ap()

    with tile.TileContext(nc) as tc:
        with (
            tc.tile_pool(name="const", bufs=1) as const_pool,
            tc.tile_pool(name="xin", bufs=3) as xin_pool,
            tc.tile_pool(name="xts", bufs=12) as xts_pool,
            tc.tile_pool(name="yout", bufs=3) as yout_pool,
            tc.tile_pool(name="xtp", bufs=2, space="PSUM") as xtp_pool,
            tc.tile_pool(name="yps", bufs=2, space="PSUM") as yps_pool,
        ):
            ident = const_pool.tile([MUL, MUL], F32)
            make_identity(nc, ident[:])
            # kern_sb: [i=128, (t,o)=384], pre-scaled by alpha
            kern_sb = const_pool.tile([MUL, 3 * MUL], F32)
            nc.sync.dma_start(
                out=kern_sb[:].rearrange("i (t o) -> i t o", t=3),
                in_=kern.rearrange("t i o -> i t o"),
            )
            nc.vector.tensor_scalar_mul(kern_sb[:], kern_sb[:], ALPHA)

            for t in range(TILES_PER_CORE):
                nbase = t * TILE_N
                x_sb = []
                for j in range(4):
                    xt_in = xin_pool.tile([MUL, DIM], F32, tag=f"xin{j}")
                    nc.sync.dma_start(
                        out=xt_in[:],
                        in_=x[nbase + j * MUL : nbase + (j + 1) * MUL, :],
                    )
                    x_sb.append(xt_in)

                # transpose all 9 m-slices: xT_m [i=128, n=512]
                xt_all = []
                for m in range(NMDIM):
                    xtp = xtp_pool.tile([MUL, TILE_N], F32, tag="xtp")
                    for j in range(4):
                        nc.tensor.transpose(
                            xtp[:, j * MUL : (j + 1) * MUL],
                            x_sb[j][:].rearrange("p (i m) -> p m i", m=NMDIM)[
                                :, m, :
                            ],
                            ident[:],
                        )
                    xt_sb = xts_pool.tile([MUL, TILE_N], F32, tag="xts")
                    if m % 2 == 0:
                        nc.vector.tensor_copy(xt_sb[:], xtp[:])
                    else:
                        nc.scalar.copy(xt_sb[:], xtp[:])
                    xt_all.append(xt_sb)

                # per n-block: 9 matmuls into one 3-bank PSUM tensor, then
                # a single strided assembly copy and the store DMA
                for j in range(4):
                    yp = yps_pool.tile([MUL, NMDIM * MUL], F32, tag="yps")
                    for m in range(NMDIM):
                        ks = IRREP_OF_M[m] * MUL
                        nc.tensor.matmul(
                            yp[:, m * MUL : (m + 1) * MUL],
                            xt_all[m][:, j * MUL : (j + 1) * MUL],
                            kern_sb[:, ks : ks + MUL],
                            start=True,
                            stop=True,
                        )
                    out_sb = yout_pool.tile(
                        [MUL, DIM], F32, tag=f"yout{j}", name=f"yout{j}"
                    )
                    # split the interleaving copy across DVE and ACT so the
                    # two halves run in parallel (shorter convoy, balanced)
                    dst = out_sb[:].rearrange("p (o m) -> p o m", o=MUL)
                    src = yp[:].rearrange("p (m o) -> p o m", o=MUL)
                    half = MUL // 2
                    nc.vector.tensor_copy(dst[:, :half, :], src[:, :half, :])
                    nc.scalar.copy(dst[:, half:, :], src[:, half:, :])
                    nc.sync.dma_start(
                        out=y[nbase + j * MUL : nbase + (j + 1) * MUL, :],
                        in_=out_sb[:],
                    )

    nc.compile()
    return nc


def kernel(**inputs):
    from concourse import bass_utils

    x = np.ascontiguousarray(np.asarray(inputs["x"], dtype=np.float32))
    kern = np.ascontiguousarray(np.asarray(inputs["kernel"], dtype=np.float32))

    if "nc" not in _cache:
        _cache["nc"] = _build()
    nc = _cache["nc"]

    xp = np.zeros((N_CORES * PER_CORE, DIM), dtype=np.float32)
    xp[:N_NODES] = x
    in_maps = [
        {"x": xp[c * PER_CORE : (c + 1) * PER_CORE], "kern": kern}
        for c in range(N_CORES)
    ]

    trace = os.environ.get("KERNEL_TRACE", "0") == "1"
    if trace:
        _install_trace_support()
    res = bass_utils.run_bass_kernel_spmd(
        nc, in_maps, core_ids=list(range(N_CORES)), trace=trace
    )
    global LAST_RESULTS
    LAST_RESULTS = res

    out = np.concatenate([res.results[c]["y"] for c in range(N_CORES)], axis=0)
    return np.ascontiguousarray(out[:N_NODES])


# revision 6
# speedup vs baseline: 1.3806x; 1.0771x over previous
"""Trainium2 Bass kernel for AlignedLinear (irreps 0e+1o+2e, mul 128).

y[n, o*9+m] = alpha * sum_i x[n, i*9+m] * K[irrep(m), i, o]

Strategy (data-parallel over nodes, 8 cores):
  - pad 50000 nodes -> 8 * 6656, each core gets 13 tiles of 512 nodes
  - per 512-node tile: load x naturally [128n x 1152], PE-transpose the 9
    strided m-slices to xT_m [128i x 512n] (fp32, exact), then for each
    (m, j-block) one fp32 matmul with xT as the stationary operand:
    out = xT_block^T @ K_irrep(m) = y_block [128n x 128o] (n-major, no
    second transpose needed), finally strided scaled copies assemble the
    interleaved [n, (o,m)] output tile and contiguous DMAs store it.
"""

import os

import numpy as np

N_NODES = 50000
DIM = 1152
MUL = 128
NMDIM = 9  # total irrep dim (1+3+5)
IRREP_OF_M = [0, 1, 1, 1, 2, 2, 2, 2, 2]
ALPHA = float(np.sqrt(1.0 / MUL))
N_CORES = 8
TILE_N = 512
TILES_PER_CORE = 13
PER_CORE = TILE_N * TILES_PER_CORE  # 6656
M_GROUPS = [[0, 1, 2, 3], [4, 5, 6, 7], [8]]

_cache = {}

LAST_RESULTS = None  # BassKernelResults of the most recent run (for test.py)


def _install_trace_support():
    """Make trace=True work under axon: inject the missing
    antenv.axon_hooks module and neuter the S3 artifact upload."""
    import contextlib
    import ctypes
    import sys
    import types

    if "antenv.axon_hooks" not in sys.modules:
        mod = types.ModuleType("antenv.axon_hooks")

        def _make_hook():
            try:
                lib = ctypes.CDLL("/opt/axon/libaxon_pjrt.so")
            except OSError:
                return None
            if not hasattr(lib, "axon_start_nrt_profile"):
                return None
            lib.axon_start_nrt_profile.argtypes = [
                ctypes.POINTER(ctypes.c_int64),
                ctypes.c_size_t,
            ]
            lib.axon_start_nrt_profile.restype = ctypes.c_int64
            lib.axon_stop_nrt_profile.argtypes = [ctypes.c_char_p]
            lib.axon_stop_nrt_profile.restype = ctypes.c_int64

            @contextlib.contextmanager
            def _hook(output_dir, device_ids):
                import jax

                jax.devices()
                if device_ids:
                    ids = (ctypes.c_int64 * len(device_ids))(*device_ids)
                    rc = lib.axon_start_nrt_profile(ids, len(device_ids))
                else:
                    rc = lib.axon_start_nrt_profile(None, 0)
                if rc != 0:
                    raise RuntimeError(f"axon_start_nrt_profile rc={rc}")
                try:
                    yield
                finally:
                    n = lib.axon_stop_nrt_profile(str(output_dir).encode())
                    print(f"ntff profile: {n} file(s) -> {output_dir}")

            return _hook

        hook = _make_hook()
        mod.get_axon_ntff_profile_hook = lambda: hook
        mod.set_axon_ntff_profile_hook = lambda h: None
        sys.modules["antenv.axon_hooks"] = mod

    import concourse.bass_utils as bass_utils

    bass_utils.upload_artifacts = lambda tmpdir: tmpdir


def _build():
    import concourse.mybir as mybir
    import concourse.tile as tile
    from concourse import bacc
    from concourse.masks import make_identity

    F32 = mybir.dt.float32

    nc = bacc.Bacc("TRN2", target_bir_lowering=False)
    x = nc.dram_tensor("x", [PER_CORE, DIM], F32, kind="ExternalInput").ap()
    kern = nc.dram_tensor("kern", [3, MUL, MUL], F32, kind="ExternalInput").ap()
    y = nc.dram_tensor("y", [PER_CORE, DIM], F32, kind="ExternalOutput").ap()

    with tile.TileContext(nc) as tc:
        with (
            tc.tile_pool(name="const", bufs=1) as const_pool,
            tc.tile_pool(name="xin", bufs=3) as xin_pool,
            tc.tile_pool(name="xts", bufs=12) as xts_pool,
            tc.tile_pool(name="yout", bufs=3) as yout_pool,
            tc.tile_pool(name="xtp", bufs=2, space="PSUM") as xtp_pool,
            tc.tile_pool(name="yps", bufs=2, space="PSUM") as yps_pool,
        ):
            ident = const_pool.tile([MUL, MUL], F32)
            make_identity(nc, ident[:])
            # kern_sb: [i=128, (t,o)=384], pre-scaled by alpha
            kern_sb = const_pool.tile([MUL, 3 * MUL], F32)
            nc.sync.dma_start(
                out=kern_sb[:].rearrange("i (t o) -> i t o", t=3),
                in_=kern.rearrange("t i o -> i t o"),
            )
            nc.vector.tensor_scalar_mul(kern_sb[:], kern_sb[:], ALPHA)

            for t in range(TILES_PER_CORE):
                nbase = t * TILE_N
                x_sb = []
                for j in range(4):
                    xt_in = xin_pool.tile([MUL, DIM], F32, tag=f"xin{j}")
                    nc.sync.dma_start(
                        out=xt_in[:],
                        in_=x[nbase + j * MUL : nbase + (j + 1) * MUL, :],
                    )
                    x_sb.append(xt_in)

                # transpose all 9 m-slices: xT_m [i=128, n=512]
                xt_all = []
                for m in range(NMDIM):
                    xtp = xtp_pool.tile([MUL, TILE_N], F32, tag="xtp")
                    for j in range(4):
                        nc.tensor.transpose(
                            xtp[:, j * MUL : (j + 1) * MUL],
                            x_sb[j][:].rearrange("p (i m) -> p m i", m=NMDIM)[
                                :, m, :
                            ],
                            ident[:],
                        )
                    xt_sb = xts_pool.tile([MUL, TILE_N], F32, tag="xts")
                    if m % 2 == 0:
                        nc.vector.tensor_copy(xt_sb[:], xtp[:])
                    else:
                        nc.scalar.copy(xt_sb[:], xtp[:])
                    xt_all.append(xt_sb)

                # per n-block: 9 matmuls into one 3-bank PSUM tensor, then
                # a single strided assembly copy and the store DMA
                for j in range(4):
                    yp = yps_pool.tile([MUL, NMDIM * MUL], F32, tag="yps")
                    for m in range(NMDIM):
                        ks = IRREP_OF_M[m] * MUL
                        nc.tensor.matmul(
                            yp[:, m * MUL : (m + 1) * MUL],
                            xt_all[m][:, j * MUL : (j + 1) * MUL],
                            kern_sb[:, ks : ks + MUL],
                            start=True,
                            stop=True,
                        )
                    out_sb = yout_pool.tile(
                        [MUL, DIM], F32, tag=f"yout{j}", name=f"yout{j}"
                    )
                    # split the interleaving copy across DVE and ACT so the
                    # two halves run in parallel (shorter convoy, balanced)
                    dst = out_sb[:].rearrange("p (o m) -> p o m", o=MUL)
                    src = yp[:].rearrange("p (m o) -> p o m", o=MUL)
                    half = MUL // 2
                    nc.vector.tensor_copy(dst[:, :half, :], src[:, :half, :])
                    nc.scalar.copy(dst[:, half:, :], src[:, half:, :])
                    # stores go out the ACT HWDGE queue so a store waiting on
                    # out_sb never blocks the next tile's x-load issue on SP
                    nc.scalar.dma_start(
                        out=y[nbase + j * MUL : nbase + (j + 1) * MUL, :],
                        in_=out_sb[:],
                    )

    nc.compile()
    return nc


def kernel(**inputs):
    from concourse import bass_utils

    x = np.ascontiguousarray(np.asarray(inputs["x"], dtype=np.float32))
    kern = np.ascontiguousarray(np.asarray(inputs["kernel"], dtype=np.float32))

    if "nc" not in _cache:
        _cache["nc"] = _build()
    nc = _cache["nc"]

    xp = np.zeros((N_CORES * PER_CORE, DIM), dtype=np.float32)
    xp[:N_NODES] = x
    in_maps = [
        {"x": xp[c * PER_CORE : (c + 1) * PER_CORE], "kern": kern}
        for c in range(N_CORES)
    ]

    trace = os.environ.get("KERNEL_TRACE", "0") == "1"
    if trace:
        _install_trace_support()
    res = bass_utils.run_bass_kernel_spmd(
        nc, in_maps, core_ids=list(range(N_CORES)), trace=trace
    )
    global LAST_RESULTS
    LAST_RESULTS = res

    out = np.concatenate([res.results[c]["y"] for c in range(N_CORES)], axis=0)
    return np.ascontiguousarray(out[:N_NODES])
